# revision 1
# baseline (speedup 1.0000x reference)
"""MoChA stable chunkwise attention (window w=16) on 8 Trainium2 NeuronCores.

The reference's stabilizing moving-max cancels algebraically:
    P[t] = exp(logits[t]);  S[u] = sum_{v=u-15..u} P[v]
    R[u] = emit[u]/S[u];    out[t] = P[t] * sum_{k=0..15} R[t+k]
Both width-16 window sums run on the TensorEngine as banded matmuls in a
transposed layout (partition = t mod 128, free = (block, chunk) columns);
cross-block window wrap is handled by corner matmuls accumulating in PSUM,
with zero-masked operands at row boundaries. The host pre-permutes all
inputs into device layout (plain contiguous DMA loads, no on-device
transposes) and un-permutes the output. Logits travel as fp16 value +
fp16 residual planes whose on-device sum restores fp32 accuracy.

Self-contained: only numpy + concourse (on PYTHONPATH) required.
"""

import numpy as np

import concourse.bass as bass
import concourse.tile as tile
import concourse.mybir as mybir
from concourse import bacc
from concourse.bass_utils import run_bass_kernel_spmd

F32 = mybir.dt.float32
F16 = mybir.dt.float16
ACTF = mybir.ActivationFunctionType

B, T = 64, 16384
NCORES = 8
RPC = B // NCORES        # 8 rows/core
NCH = 16                 # chunks per row
CH = 1024                # elems per chunk
NPART = 128
NBLK = CH // 128         # 8 blocks per chunk
W = 16                   # window
NF = RPC * T // 128      # 1024 layout-B columns


def make_consts():
    k = np.arange(128)[:, None]
    m = np.arange(128)[None, :]
    band0 = (m - k >= 0) & (m - k <= W - 1)            # S within-block
    corner = (k - m >= 128 - W + 1) & (k - m <= 127)   # S from prev block
    banda = (k - m >= 0) & (k - m <= W - 1)            # Z within-block
    cornera = (m - k >= 128 - W + 1) & (m - k <= 127)  # Z from next block
    return np.concatenate(
        [x.astype(np.float16) for x in (band0, corner, banda, cornera)],
        axis=1,
    )  # [128, 512]


def _perm(a):
    """[RPC, T] -> layout B [128, NF]: full host-side transpose, so device
    loads are plain contiguous [128 partitions x NF] DMAs (no xbar)."""
    return np.ascontiguousarray(
        a.reshape(RPC, NCH, NBLK, 128).transpose(3, 2, 0, 1).reshape(128, NF)
    )


def unperm_out(o):
    """[128, NF] layout B -> [RPC, T]."""
    return np.ascontiguousarray(
        o.reshape(128, NBLK, RPC, NCH)
        .transpose(2, 3, 1, 0)
        .reshape(RPC, T)
    )


def build_nc():
    nc = bacc.Bacc("TRN2", target_bir_lowering=False, debug=False,
                   num_devices=NCORES)
    hi_t = nc.dram_tensor("lg_hi", [NPART, NF], F16, kind="ExternalInput")
    lo_t = nc.dram_tensor("lg_lo", [NPART, NF], F16, kind="ExternalInput")
    em_t = nc.dram_tensor("em16", [NPART, NF], F16, kind="ExternalInput")
    kc_t = nc.dram_tensor("consts16", [NPART, 512], F16, kind="ExternalInput")
    out_t = nc.dram_tensor("out", [NPART, NF], F32, kind="ExternalOutput")

    H1 = slice(512, 1024)
    H0 = slice(0, 512)

    with tile.TileContext(nc) as tc:
        with (
            tc.tile_pool(name="sb", bufs=1) as sb,
            tc.tile_pool(name="ps", bufs=1, space="PSUM") as ps,
        ):
            kb = sb.tile([NPART, 512], F16, tag="kb")
            hi_b = sb.tile([NPART, CH], F16, tag="hi_b")
            lo_b = sb.tile([NPART, CH], F16, tag="lo_b")
            lg_b = sb.tile([NPART, CH], F32, tag="lg_b")
            e_b = sb.tile([NPART, CH], F16, tag="e_b")
            p_b = sb.tile([NPART, CH], F16, tag="p_b")
            rcp_b = sb.tile([NPART, CH], F32, tag="rcp_b")
            r_b = sb.tile([NPART, CH], F16, tag="r_b")
            o_b = sb.tile([NPART, CH], F32, tag="o_b")

            pz_b = sb.tile([NPART, 129], F16, tag="pz_b")
            rz_b = sb.tile([NPART, 129], F16, tag="rz_b")
            s_ps = ps.tile([NPART, CH], F32, tag="s")
            z_ps = ps.tile([NPART, CH], F32, tag="z")

            band0 = kb[:, 0:128]
            corner = kb[:, 128:256]
            banda = kb[:, 256:384]
            cornera = kb[:, 384:512]

            # ---- loads: all plain contiguous DMAs, spread over both HWDGE;
            # h1 planes first, h0 planes right behind, S-consts between,
            # Z-consts and emit last (needed latest) ----
            nc.sync.dma_start(
                hi_b[:, 512:1024], bass.AP(hi_t, 512, [[NF, NPART], [1, 512]]))
            nc.scalar.dma_start(
                lo_b[:, 512:1024], bass.AP(lo_t, 512, [[NF, NPART], [1, 512]]))
            nc.sync.dma_start(
                hi_b[:, 0:512], bass.AP(hi_t, 0, [[NF, NPART], [1, 512]]))
            nc.scalar.dma_start(
                lo_b[:, 0:512], bass.AP(lo_t, 0, [[NF, NPART], [1, 512]]))
            nc.sync.dma_start(kb[:, 0:256],
                              bass.AP(kc_t, 0, [[512, NPART], [1, 256]]))
            nc.scalar.dma_start(
                e_b[:, :], bass.AP(em_t, 0, [[NF, NPART], [1, NF]]))
            nc.sync.dma_start(kb[:, 256:512],
                              bass.AP(kc_t, 256, [[512, NPART], [1, 256]]))

            # ---- logits = hi + lo (fp32), exp -> fp16 P; h1 first (the S
            # block-0 corner reads block 7) ----
            for h in (H1, H0):
                nc.vector.tensor_add(lg_b[:, h], hi_b[:, h], lo_b[:, h])
                nc.scalar.activation(p_b[:, h], lg_b[:, h], ACTF.Exp)

            # masked wrap operand for S block 0: pz[:, j] = p_b[:, 896+j-1],
            # zeroed at j==0 and j%16==0 (row starts)
            nc.scalar.copy(pz_b[:, 1:128], p_b[:, 896:1023])
            nc.vector.memset(pz_b[:, 0:1], 0.0)
            for rr in range(1, RPC):
                nc.vector.memset(pz_b[:, 16 * rr:16 * rr + 1], 0.0)

            # ---- S matmuls: one N=512 band per half, corners per block
            # (each closing its block's accumulation group) ----
            def s_corner(b):
                sl = slice(b * 128, (b + 1) * 128)
                rhs = pz_b[:, 0:128] if b == 0 else p_b[:, (b - 1) * 128:b * 128]
                nc.tensor.matmul(s_ps[:, sl], corner, rhs,
                                 start=False, stop=True, skip_group_check=True)

            nc.tensor.matmul(s_ps[:, H1], band0, p_b[:, H1],
                             start=True, stop=False, skip_group_check=True)
            for b in (5, 6, 7):
                s_corner(b)
            nc.tensor.matmul(s_ps[:, H0], band0, p_b[:, H0],
                             start=True, stop=False, skip_group_check=True)
            for b in (0, 1, 2, 3, 4):
                s_corner(b)

            # ---- 1/S ----
            for h in (H0, H1):
                nc.vector.reciprocal_approx_fast(rcp_b[:, h], s_ps[:, h])
            # ---- R = emit * (1/S); h1 on the idle Pool engine so both
            # halves finish together and Z unblocks earlier ----
            nc.gpsimd.tensor_mul(r_b[:, H1], e_b[:, H1], rcp_b[:, H1])
            nc.vector.tensor_mul(r_b[:, H0], e_b[:, H0], rcp_b[:, H0])

            # masked wrap operand for Z block 7: rz[:, 1:129] streams
            # r_b[:, 1:128]+pad; row-start cols (j%16==0) and col 128 zero
            nc.scalar.copy(rz_b[:, 1:128], r_b[:, 1:128])
            nc.vector.memset(rz_b[:, 128:129], 0.0)
            for rr in range(1, RPC):
                nc.vector.memset(rz_b[:, 16 * rr:16 * rr + 1], 0.0)

            # ---- Z matmuls: one N=512 band per half, corners per block ----
            def z_corner(b):
                sl = slice(b * 128, (b + 1) * 128)
                rhs = (rz_b[:, 1:129] if b == NBLK - 1
                       else r_b[:, (b + 1) * 128:(b + 2) * 128])
                nc.tensor.matmul(z_ps[:, sl], cornera, rhs,
                                 start=False, stop=True, skip_group_check=True)

            nc.tensor.matmul(z_ps[:, H0], banda, r_b[:, H0],
                             start=True, stop=False, skip_group_check=True)
            for b in (0, 1, 2):
                z_corner(b)
            nc.tensor.matmul(z_ps[:, H1], banda, r_b[:, H1],
                             start=True, stop=False, skip_group_check=True)
            for b in (3, 4, 5, 6, 7):
                z_corner(b)

            # ---- out = P * Z (fp32), store directly in layout B ----
            nc.vector.tensor_mul(o_b[:, H0], p_b[:, H0], z_ps[:, H0])
            nc.vector.tensor_mul(o_b[:, H1], p_b[:, H1], z_ps[:, H1])
            nc.sync.dma_start(
                bass.AP(out_t, 0, [[NF, NPART], [1, 512]]), o_b[:, H0])
            nc.scalar.dma_start(
                bass.AP(out_t, 512, [[NF, NPART], [1, 512]]), o_b[:, H1])

    nc.compile()
    return nc


def make_in_maps(emit_probs, softmax_logits):
    lg = np.asarray(softmax_logits, dtype=np.float32)
    hi = lg.astype(np.float16)
    lo = (lg - hi.astype(np.float32)).astype(np.float16)
    em16 = np.asarray(emit_probs, dtype=np.float16)
    consts = make_consts()
    maps = []
    for k in range(NCORES):
        rows = slice(k * RPC, (k + 1) * RPC)
        maps.append({
            "lg_hi": _perm(hi[rows]),
            "lg_lo": _perm(lo[rows]),
            "em16": _perm(em16[rows]),
            "consts16": consts,
        })
    return maps


_NC_CACHE = None


def _get_nc():
    global _NC_CACHE
    if _NC_CACHE is None:
        _NC_CACHE = build_nc()
    return _NC_CACHE


def run(emit_probs, softmax_logits, trace=False, **kwargs):
    nc = _get_nc()
    in_maps = make_in_maps(emit_probs, softmax_logits)
    res = run_bass_kernel_spmd(
        nc, in_maps, core_ids=list(range(NCORES)), trace=trace, **kwargs
    )
    out = np.concatenate(
        [unperm_out(res.results[k]["out"]) for k in range(NCORES)], axis=0
    )
    return out, res


def kernel(emit_probs, softmax_logits):
    return run(emit_probs, softmax_logits)[0]



# revision 2
# speedup vs baseline: 1.1593x; 1.1593x over previous
"""MoChA stable chunkwise attention (window w=16) on 8 Trainium2 NeuronCores.

The reference's stabilizing moving-max cancels algebraically:
    P[t] = exp(logits[t]);  S[u] = sum_{v=u-15..u} P[v]
    R[u] = emit[u]/S[u];    out[t] = P[t] * sum_{k=0..15} R[t+k]
Both width-16 window sums run on the TensorEngine as banded matmuls in a
transposed layout: partition = t mod 128, column = (row, chunk', block)
with the BLOCK index innermost, so the cross-block window wrap is a plain
+-1-column shift of the rhs AP. One guard chunk (ch'=0) per row absorbs
row boundaries: the host plants lg=-30 (exp -> 0) there, and the R guard
columns are memset to 0 once (rmul writes only real columns).  Everything
travels fp16 (logits, emit, output); the host casts the output to fp32.

Self-contained: only numpy + concourse (on PYTHONPATH) required.
"""

import numpy as np

import concourse.bass as bass
import concourse.tile as tile
import concourse.mybir as mybir
from concourse import bacc
from concourse.bass_utils import run_bass_kernel_spmd

F32 = mybir.dt.float32
F16 = mybir.dt.float16
ACTF = mybir.ActivationFunctionType

B, T = 64, 16384
NCORES = 8
RPC = B // NCORES        # 8 rows/core
NCH = 16                 # real chunks per row
CHP = NCH + 1            # +1 guard chunk (ch'=0)
NBLK = 8                 # blocks per chunk (innermost col index)
NPART = 128
W = 16
NFG = RPC * CHP * NBLK   # 1088 device columns
RB = CHP * NBLK          # 136 cols per row
HA, HB = 544, NFG - 544  # half split at row 3/4 boundary (rows 0-3 | 4-7)
GUARD_LG = -30.0


def make_consts():
    k = np.arange(128)[:, None]
    m = np.arange(128)[None, :]
    band0 = (m - k >= 0) & (m - k <= W - 1)            # S within-block
    corner = (k - m >= 128 - W + 1) & (k - m <= 127)   # S from prev col (-1)
    banda = (k - m >= 0) & (k - m <= W - 1)            # Z within-block
    cornera = (m - k >= 128 - W + 1) & (m - k <= 127)  # Z from next col (+1)
    return np.concatenate(
        [x.astype(np.float16) for x in (band0, corner, banda, cornera)],
        axis=1,
    )  # [128, 512]


def _perm(a, guard_fill):
    """[RPC, T] -> [128, NFG], col = (r*CHP + ch')*NBLK + blk, ch'=0 guard."""
    t = a.reshape(RPC, NCH, NBLK, 128).transpose(3, 0, 1, 2)  # [p, r, ch, blk]
    g = np.full((128, RPC, 1, NBLK), guard_fill, t.dtype)
    return np.ascontiguousarray(
        np.concatenate([g, t], axis=2).reshape(128, NFG)
    )


def unperm_out(o):
    """[128, NFG] -> [RPC, T] (drop guard chunks)."""
    t = o.reshape(128, RPC, CHP, NBLK)[:, :, 1:, :]  # [p, r, ch, blk]
    return np.ascontiguousarray(
        t.transpose(1, 2, 3, 0).reshape(RPC, T)
    )


def build_nc():
    nc = bacc.Bacc("TRN2", target_bir_lowering=False, debug=False,
                   num_devices=NCORES)
    lg_t = nc.dram_tensor("lg16", [NPART, NFG], F16, kind="ExternalInput")
    em_t = nc.dram_tensor("em16", [NPART, NFG], F16, kind="ExternalInput")
    kc_t = nc.dram_tensor("consts16", [NPART, 512], F16, kind="ExternalInput")
    out_t = nc.dram_tensor("out16", [NPART, NFG], F16, kind="ExternalOutput")

    A = slice(0, HA)            # rows 0-3
    Bh = slice(HA, NFG)         # rows 4-7

    with tile.TileContext(nc) as tc:
        with (
            tc.tile_pool(name="sb", bufs=1) as sb,
            tc.tile_pool(name="ps", bufs=1, space="PSUM") as ps,
        ):
            kb = sb.tile([NPART, 512], F16, tag="kb")
            lg_b = sb.tile([NPART, NFG], F16, tag="lg_b")
            e_b = sb.tile([NPART, NFG], F16, tag="e_b")
            p_b = sb.tile([NPART, NFG], F16, tag="p_b")
            rcp_b = sb.tile([NPART, NFG], F32, tag="rcp_b")
            r_b = sb.tile([NPART, NFG + 8], F16, tag="r_b")  # +8 pad cols
            o_b = sb.tile([NPART, NFG], F16, tag="o_b")
            s_ps = ps.tile([NPART, NFG], F32, tag="s")
            z_ps = ps.tile([NPART, NFG], F32, tag="z")

            band0 = kb[:, 0:128]
            corner = kb[:, 128:256]
            banda = kb[:, 256:384]
            cornera = kb[:, 384:512]

            # ---- loads: half A first on sync ring, consts first on scalar
            nc.sync.dma_start(
                lg_b[:, A], bass.AP(lg_t, 0, [[NFG, NPART], [1, HA]]))
            nc.scalar.dma_start(
                kb[:, :], bass.AP(kc_t, 0, [[512, NPART], [1, 512]]))
            nc.sync.dma_start(
                lg_b[:, Bh], bass.AP(lg_t, HA, [[NFG, NPART], [1, HB]]))
            nc.scalar.dma_start(
                e_b[:, :], bass.AP(em_t, 0, [[NFG, NPART], [1, NFG]]))

            # guard + pad columns of r_b zeroed once (rmul writes only real
            # cols, so this has no upstream dependency): cols [136r, 136r+8)
            # for r=0..7 plus the 8 pad cols at 1088.
            rb_ap = r_b[:, 0:NFG + 8]
            guards = bass.AP(
                rb_ap.tensor, rb_ap.offset, [rb_ap.ap[0], [RB, 9], [1, 8]])
            nc.vector.memset(guards, 0.0)

            def mm(out, lhsT, rhs, start, stop):
                nc.tensor.matmul(out, lhsT, rhs, start=start, stop=stop,
                                 skip_group_check=True)

            def s_half(lo, hi):
                # band (start) then corner rhs shifted -1 col (stop); the
                # half's first column is a guard (or col 0) -> band-only.
                mid = min(lo + 512 - lo % 512 if lo % 512 else lo + 512, hi)
                for a, b in ((lo, mid), (mid, hi)):
                    if a < b:
                        mm(s_ps[:, a:b], band0, p_b[:, a:b], True, False)
                for a, b in ((lo + 1, mid), (mid, hi)):
                    if a < b:
                        mm(s_ps[:, a:b], corner, p_b[:, a - 1:b - 1],
                           False, True)

            def z_half(lo, hi):
                mid = min(lo + 512 - lo % 512 if lo % 512 else lo + 512, hi)
                for a, b in ((lo, mid), (mid, hi)):
                    if a < b:
                        mm(z_ps[:, a:b], banda, r_b[:, a:b], True, False)
                for a, b in ((lo, mid), (mid, hi)):
                    if a < b:
                        mm(z_ps[:, a:b], cornera, r_b[:, a + 1:b + 1],
                           False, True)

            def real_cols(t3, half):
                # 3D AP over the 4 rows of a half, skipping the 8 guard cols
                ap = t3[:, 0:NFG]
                base = 0 if half == 0 else HA
                return bass.AP(ap.tensor, ap.offset + base + 8,
                               [ap.ap[0], [RB, 4], [1, RB - 8]])

            # ---- half A ----
            nc.scalar.activation(p_b[:, A], lg_b[:, A], ACTF.Exp)
            s_half(0, HA)
            nc.vector.reciprocal_approx_fast(rcp_b[:, A], s_ps[:, A])
            nc.vector.tensor_mul(real_cols(r_b, 0), real_cols(e_b, 0),
                                 real_cols(rcp_b, 0))

            # ---- half B ----
            nc.scalar.activation(p_b[:, Bh], lg_b[:, Bh], ACTF.Exp)
            s_half(HA, NFG)
            nc.vector.reciprocal_approx_fast(rcp_b[:, Bh], s_ps[:, Bh])

            # ---- Z + out, half A ----
            z_half(0, HA)
            nc.vector.tensor_mul(o_b[:, A], p_b[:, A], z_ps[:, A])
            nc.sync.dma_start(
                bass.AP(out_t, 0, [[NFG, NPART], [1, HA]]), o_b[:, A])

            # ---- Z + out, half B ----
            nc.vector.tensor_mul(real_cols(r_b, 1), real_cols(e_b, 1),
                                 real_cols(rcp_b, 1))
            z_half(HA, NFG)
            nc.vector.tensor_mul(o_b[:, Bh], p_b[:, Bh], z_ps[:, Bh])
            nc.scalar.dma_start(
                bass.AP(out_t, HA, [[NFG, NPART], [1, HB]]), o_b[:, Bh])

    nc.compile()
    return nc


def make_in_maps(emit_probs, softmax_logits):
    lg16 = np.asarray(softmax_logits, dtype=np.float16)
    em16 = np.asarray(emit_probs, dtype=np.float16)
    consts = make_consts()
    maps = []
    for k in range(NCORES):
        rows = slice(k * RPC, (k + 1) * RPC)
        maps.append({
            "lg16": _perm(lg16[rows], np.float16(GUARD_LG)),
            "em16": _perm(em16[rows], np.float16(0.0)),
            "consts16": consts,
        })
    return maps


_NC_CACHE = None


def _get_nc():
    global _NC_CACHE
    if _NC_CACHE is None:
        _NC_CACHE = build_nc()
    return _NC_CACHE


def run(emit_probs, softmax_logits, trace=False, **kwargs):
    nc = _get_nc()
    in_maps = make_in_maps(emit_probs, softmax_logits)
    res = run_bass_kernel_spmd(
        nc, in_maps, core_ids=list(range(NCORES)), trace=trace, **kwargs
    )
    out = np.concatenate(
        [unperm_out(res.results[k]["out16"]) for k in range(NCORES)], axis=0
    ).astype(np.float32)
    return out, res


def kernel(emit_probs, softmax_logits):
    return run(emit_probs, softmax_logits)[0]
